# revision 3
# baseline (speedup 1.0000x reference)
"""Distributed attention kernel for 8 TRN2 NeuronCores.

Problem: B=2, S=2048, D=1024, H=16 heads (hd=64), no causal mask, no
scaling.  out = softmax(x@Wq (x@Wk)^T) (x@Wv) @ Wp + biases.

Sharding: DP=2 over batch x TP=4 over heads.  Core c handles batch c//4
and heads 4*(c%4) .. 4*(c%4)+3.  Each core computes its 4 heads'
attention plus the partial c_proj (rows of w_proj for its heads), then a
ReduceScatter(add) over its 4-core group yields each core's 512-row slice
of the final output.  The host reassembles the full [2,2048,1024] output.

All matmuls run in bf16 (inputs converted host-side); accumulation f32.
Softmax skips the max-subtraction (scores are O(+-20), exp is safe in
f32) so probs = exp(s) / sum exp(s); the denominator comes free as the
65th row of the PV matmul via an appended ones-column on V.
"""

import sys

if "/opt/trn_rl_repo" not in sys.path:
    sys.path.insert(0, "/opt/trn_rl_repo")

import numpy as np
import ml_dtypes

import concourse.bass as bass
import concourse.mybir as mybir
from concourse import bacc
from concourse.tile import TileContext
from concourse.bass_utils import run_bass_kernel_spmd

BF16 = mybir.dt.bfloat16
F32 = mybir.dt.float32

B, S, D = 2, 2048, 1024
H = 16
HD = 64
TP = 4  # tensor-parallel group size (cores per batch)
HPC = H // TP  # heads per core = 4
QC = HPC * HD  # q (or k or v) columns per core = 256
SQB = 512  # sq chunk (free dim of scores/pv matmuls)
NJ = S // SQB  # 4 chunks
NT = S // 128  # 16 sk tiles
NK = D // 128  # 8 contraction tiles for the projections

_CACHE = {}


def build():
    nc = bacc.Bacc(num_devices=8)

    xT_ext = nc.declare_dram_parameter("xT", [D, S], BF16, isOutput=False)
    wqkv_ext = nc.declare_dram_parameter("wqkv", [D, 3 * QC], BF16, isOutput=False)
    bqk_ext = nc.declare_dram_parameter("bqk", [2 * QC, 1], F32, isOutput=False)
    bv_ext = nc.declare_dram_parameter("bv", [1, QC], BF16, isOutput=False)
    wpa_ext = nc.declare_dram_parameter("wpa", [QC + 1, D], BF16, isOutput=False)
    out_ext = nc.declare_dram_parameter("out", [S // TP, D], F32, isOutput=True)

    partial = nc.dram_tensor("partial", [S, D], F32)
    rs_out = nc.dram_tensor("rs_out", [S // TP, D], F32)

    with TileContext(nc) as tc:
        with (
            tc.tile_pool(name="persist", bufs=1) as persist,
            tc.tile_pool(name="expt_pool", bufs=2) as expt_pool,
            tc.tile_pool(name="mm", bufs=4, space="PSUM") as mm_pool,
            tc.tile_pool(name="pv", bufs=2, space="PSUM") as pv_pool,
            tc.tile_pool(name="small", bufs=4) as small_pool,
            tc.tile_pool(name="ot", bufs=8) as ot_pool,
            tc.tile_pool(name="osb", bufs=4) as osb_pool,
        ):
            # ---- load persistent tiles ----
            xt = []
            for k in range(NK):
                t = persist.tile([128, S], BF16, tag=f"xt{k}", name=f"xt{k}")
                nc.sync.dma_start(out=t, in_=xT_ext[k * 128 : (k + 1) * 128, :])
                xt.append(t)
            wt = []
            for k in range(NK):
                t = persist.tile([128, 3 * QC], BF16, tag=f"wt{k}", name=f"wt{k}")
                nc.sync.dma_start(out=t, in_=wqkv_ext[k * 128 : (k + 1) * 128, :])
                wt.append(t)
            wp = []
            for h in range(HPC):
                t = persist.tile([HD, D], BF16, tag=f"wp{h}", name=f"wp{h}")
                nc.sync.dma_start(out=t, in_=wpa_ext[h * HD : (h + 1) * HD, :])
                wp.append(t)
            wp_bias = persist.tile([1, D], BF16, tag="wpb", name="wpb")
            nc.sync.dma_start(out=wp_bias, in_=wpa_ext[2 * 128 : 2 * 128 + 1, :])
            bqk = []
            for k in range(4):
                t = persist.tile([128, 1], F32, tag=f"bqk{k}", name=f"bqk{k}")
                nc.sync.dma_start(out=t, in_=bqk_ext[k * 128 : (k + 1) * 128, :])
                bqk.append(t)
            bv = persist.tile([1, QC], BF16, tag="bv", name="bv")
            nc.sync.dma_start(out=bv, in_=bv_ext[:, :])
            ones_row = persist.tile([1, 128], BF16, tag="ones", name="ones")
            nc.vector.memset(ones_row, 1.0)

            # ---- QKV projection ----
            # q/k transposed layout: qk_sb[ct] [128, S], ct 0-1 = q cols,
            # ct 2-3 = k cols; head h lives on partitions (h%2)*64 of
            # tile h//2 (+2 for k).
            qk_sb = []
            for ct in range(4):
                t = persist.tile([128, S], BF16, tag=f"qk{ct}", name=f"qk{ct}")
                qk_sb.append(t)
            for ct in range(4):
                for ns in range(NJ):
                    ps = mm_pool.tile([128, SQB], F32, tag="mm", name="ps_qkv")
                    for k in range(NK):
                        nc.tensor.matmul(
                            ps,
                            wt[k][:, ct * 128 : (ct + 1) * 128],
                            xt[k][:, ns * SQB : (ns + 1) * SQB],
                            start=(k == 0),
                            stop=(k == NK - 1),
                        )
                    nc.scalar.activation(
                        qk_sb[ct][:, ns * SQB : (ns + 1) * SQB],
                        ps,
                        mybir.ActivationFunctionType.Identity,
                        bias=bqk[ct],
                    )

            # v natural layout + ones column: v_sb[t] [128, HPC, 65];
            # [:, h, :64] = v for head h, [:, h, 64] = 1.0
            v_sb = []
            for t_i in range(NT):
                t = persist.tile([128, HPC, HD + 1], BF16, tag=f"v{t_i}", name=f"v{t_i}")
                v_sb.append(t)
            for t_i in range(NT):
                psv = mm_pool.tile([128, QC], F32, tag="mm", name="ps_v")
                for k in range(NK):
                    nc.tensor.matmul(
                        psv,
                        xt[k][:, t_i * 128 : (t_i + 1) * 128],
                        wt[k][:, 2 * QC : 3 * QC],
                        start=(k == 0),
                        stop=False,
                    )
                nc.tensor.matmul(psv, ones_row, bv, start=False, stop=True)
                nc.vector.memset(v_sb[t_i][:, :, HD : HD + 1], 1.0)
                for h in range(HPC):
                    nc.vector.tensor_copy(
                        v_sb[t_i][:, h, 0:HD], psv[:, h * HD : (h + 1) * HD]
                    )

            # ---- attention + c_proj, chunk by chunk over sq ----
            for j in range(NJ):
                oT = []
                for h in range(HPC):
                    qslice = qk_sb[h // 2][
                        (h % 2) * HD : (h % 2) * HD + HD, j * SQB : (j + 1) * SQB
                    ]
                    krow = qk_sb[2 + h // 2][(h % 2) * HD : (h % 2) * HD + HD, :]
                    expt = expt_pool.tile([128, NT, SQB], BF16, tag="expt", name="expt")
                    for t_i in range(NT):
                        ps_s = mm_pool.tile([128, SQB], F32, tag="mm", name="ps_s")
                        nc.tensor.matmul(
                            ps_s,
                            krow[:, t_i * 128 : (t_i + 1) * 128],
                            qslice,
                            start=True,
                            stop=True,
                        )
                        nc.scalar.activation(
                            expt[:, t_i, :], ps_s, mybir.ActivationFunctionType.Exp
                        )
                    pv = pv_pool.tile([HD + 1, SQB], F32, tag="pv", name="pv")
                    for t_i in range(NT):
                        nc.tensor.matmul(
                            pv,
                            v_sb[t_i][:, h, :],
                            expt[:, t_i, :],
                            start=(t_i == 0),
                            stop=(t_i == NT - 1),
                        )
                    rz = small_pool.tile([1, SQB], F32, tag="rz", name="rz")
                    nc.vector.reciprocal(rz, pv[HD : HD + 1, :])
                    bc = small_pool.tile([HD, SQB], F32, tag="bc", name="bc")
                    nc.gpsimd.partition_broadcast(bc, rz)
                    o = ot_pool.tile([HD, SQB], BF16, tag="ot", name=f"ot{h}")
                    nc.vector.tensor_mul(o, pv[0:HD, :], bc)
                    oT.append(o)
                # c_proj on this chunk: rows j*SQB .. +SQB of partial
                for m in range(SQB // 128):
                    for nch in range(2):
                        pc = mm_pool.tile([128, 512], F32, tag="mm", name="pc")
                        for h in range(HPC):
                            nc.tensor.matmul(
                                pc,
                                oT[h][:, m * 128 : (m + 1) * 128],
                                wp[h][:, nch * 512 : (nch + 1) * 512],
                                start=(h == 0),
                                stop=False,
                            )
                        nc.tensor.matmul(
                            pc,
                            ones_row,
                            wp_bias[:, nch * 512 : (nch + 1) * 512],
                            start=False,
                            stop=True,
                        )
                        osb = osb_pool.tile([128, 512], F32, tag="osb", name="osb")
                        nc.vector.tensor_copy(osb, pc)
                        nc.sync.dma_start(
                            out=partial[
                                j * SQB + m * 128 : j * SQB + (m + 1) * 128,
                                nch * 512 : (nch + 1) * 512,
                            ],
                            in_=osb,
                        )

            # ---- reduce-scatter over the 4-core group, then emit ----
            nc.gpsimd.collective_compute(
                "ReduceScatter",
                mybir.AluOpType.add,
                replica_groups=[[0, 1, 2, 3], [4, 5, 6, 7]],
                ins=[partial.ap()],
                outs=[rs_out.ap()],
            )
            nc.sync.dma_start(out=out_ext[:, :], in_=rs_out[:, :])

    nc.compile()
    return nc


def make_in_maps(x, w_attn, b_attn, w_proj, b_proj):
    bf = ml_dtypes.bfloat16
    in_maps = []
    for c in range(8):
        b = c // TP
        g = c % TP
        cs = slice(g * QC, (g + 1) * QC)
        xT = np.ascontiguousarray(x[b].T).astype(bf)
        wqkv = np.concatenate(
            [w_attn[:, cs], w_attn[:, D:][:, cs], w_attn[:, 2 * D :][:, cs]], axis=1
        ).astype(bf)
        bqk = np.concatenate([b_attn[cs], b_attn[D:][cs]]).reshape(2 * QC, 1)
        bqk = np.ascontiguousarray(bqk, dtype=np.float32)
        bv = np.ascontiguousarray(
            b_attn[2 * D :][cs].reshape(1, QC).astype(bf)
        )
        wpa = np.concatenate(
            [w_proj[cs, :], (b_proj / TP).reshape(1, D)], axis=0
        ).astype(bf)
        in_maps.append(
            {"xT": xT, "wqkv": wqkv, "bqk": bqk, "bv": bv, "wpa": wpa}
        )
    return in_maps


def assemble(results):
    out = np.empty((B, S, D), np.float32)
    for c in range(8):
        b = c // TP
        g = c % TP
        out[b, g * (S // TP) : (g + 1) * (S // TP), :] = results[c]["out"]
    return out


def kernel(x, w_attn, b_attn, w_proj, b_proj):
    x = np.asarray(x, dtype=np.float32)
    w_attn = np.asarray(w_attn, dtype=np.float32)
    b_attn = np.asarray(b_attn, dtype=np.float32)
    w_proj = np.asarray(w_proj, dtype=np.float32)
    b_proj = np.asarray(b_proj, dtype=np.float32)
    if "nc" not in _CACHE:
        _CACHE["nc"] = build()
    nc = _CACHE["nc"]
    in_maps = make_in_maps(x, w_attn, b_attn, w_proj, b_proj)
    res = run_bass_kernel_spmd(nc, in_maps, core_ids=list(range(8)))
    return assemble(res.results)


# revision 5
# speedup vs baseline: 1.4674x; 1.4674x over previous
"""Distributed attention kernel for 8 TRN2 NeuronCores.

Problem: B=2, S=2048, D=1024, H=16 heads (hd=64), no causal mask, no
scaling.  out = softmax((x@Wq) (x@Wk)^T) (x@Wv) @ Wp + biases.

Sharding: DP=2 over batch x TP=4 over heads.  Core c handles batch c//4
and heads 4*(c%4) .. 4*(c%4)+3.  Each core computes its 4 heads'
attention plus the partial c_proj (rows of w_proj for its heads), then a
chunked bf16 ReduceScatter(add) over its 4-core group yields each core's
512-row slice of the final output.  The host reassembles [2,2048,1024].

All matmuls run in bf16 (inputs converted host-side); accumulation f32.
Softmax skips the max-subtraction (scores are O(+-20), exp is safe in
f32): probs = exp(s) / sum exp(s); the denominator comes free as the
65th row of the PV matmul via an appended ones-column on V.
"""

import sys

if "/opt/trn_rl_repo" not in sys.path:
    sys.path.insert(0, "/opt/trn_rl_repo")

import numpy as np
import ml_dtypes

import concourse.bass as bass
import concourse.mybir as mybir
from concourse import bacc
from concourse.tile import TileContext
from concourse.bass_utils import run_bass_kernel_spmd

BF16 = mybir.dt.bfloat16
F32 = mybir.dt.float32

B, S, D = 2, 2048, 1024
H = 16
HD = 64
TP = 4  # tensor-parallel group size (cores per batch)
HPC = H // TP  # heads per core = 4
QC = HPC * HD  # q (or k or v) columns per core = 256
SQB = 512  # sq chunk (free dim of scores/pv matmuls)
NJ = S // SQB  # 4 chunks
NT = S // 128  # 16 sk tiles
NK = D // 128  # 8 contraction tiles for the projections
SO = S // TP  # 512 output rows per core

_CACHE = {}


def build():
    nc = bacc.Bacc(num_devices=8)

    xT_ext = nc.declare_dram_parameter("xT", [D, S], BF16, isOutput=False)
    wqkv_ext = nc.declare_dram_parameter("wqkv", [D, 3 * QC], BF16, isOutput=False)
    bqk_ext = nc.declare_dram_parameter("bqk", [2 * QC, 1], F32, isOutput=False)
    bv_ext = nc.declare_dram_parameter("bv", [1, QC], BF16, isOutput=False)
    wpa_ext = nc.declare_dram_parameter("wpa", [QC + 1, D], BF16, isOutput=False)
    out_ext = nc.declare_dram_parameter("out", [SO, D], F32, isOutput=True)

    partial = nc.dram_tensor("partial", [S, D], BF16)
    rs_out = [nc.dram_tensor(f"rs_out{j}", [SQB // TP, D], BF16) for j in range(NJ)]

    with TileContext(nc) as tc:
        with (
            tc.tile_pool(name="persist", bufs=1) as persist,
            tc.tile_pool(name="expt_pool", bufs=2) as expt_pool,
            tc.tile_pool(name="mm", bufs=3, space="PSUM") as mm_pool,
            tc.tile_pool(name="pv", bufs=2, space="PSUM") as pv_pool,
            tc.tile_pool(name="small", bufs=4) as small_pool,
            tc.tile_pool(name="ot", bufs=4) as ot_pool,
            tc.tile_pool(name="osb", bufs=6) as osb_pool,
        ):
            # ---- load persistent tiles ----
            xt = []
            for k in range(NK):
                t = persist.tile([128, S], BF16, tag=f"xt{k}", name=f"xt{k}")
                nc.sync.dma_start(out=t, in_=xT_ext[k * 128 : (k + 1) * 128, :])
                xt.append(t)
            wt = []
            for k in range(NK):
                t = persist.tile([128, 3 * QC], BF16, tag=f"wt{k}", name=f"wt{k}")
                nc.sync.dma_start(out=t, in_=wqkv_ext[k * 128 : (k + 1) * 128, :])
                wt.append(t)
            wp = []
            for p in range(2):
                t = persist.tile([128, D], BF16, tag=f"wp{p}", name=f"wp{p}")
                nc.sync.dma_start(out=t, in_=wpa_ext[p * 128 : (p + 1) * 128, :])
                wp.append(t)
            wp_bias = persist.tile([1, D], BF16, tag="wpb", name="wpb")
            nc.sync.dma_start(out=wp_bias, in_=wpa_ext[2 * 128 : 2 * 128 + 1, :])
            bqk = []
            for k in range(4):
                t = persist.tile([128, 1], F32, tag=f"bqk{k}", name=f"bqk{k}")
                nc.sync.dma_start(out=t, in_=bqk_ext[k * 128 : (k + 1) * 128, :])
                bqk.append(t)
            bv = persist.tile([1, QC], BF16, tag="bv", name="bv")
            nc.sync.dma_start(out=bv, in_=bv_ext[:, :])
            ones_row = persist.tile([1, 128], BF16, tag="ones", name="ones")
            nc.vector.memset(ones_row, 1.0)

            # ---- QKV projection ----
            # q/k transposed layout: qk_sb[ct] [128, S], ct 0-1 = q cols,
            # ct 2-3 = k cols; head h lives on partitions (h%2)*64 of
            # tile h//2 (+2 for k).  Emit k first so attention can start
            # before the q tiles for later chunks are done.
            qk_sb = [
                persist.tile([128, S], BF16, tag=f"qk{ct}", name=f"qk{ct}")
                for ct in range(4)
            ]
            for ct in (2, 3, 0, 1):
                for ns in range(NJ):
                    ps = mm_pool.tile([128, 2, SQB], F32, tag="mm", name="ps_qkv")
                    for k in range(NK):
                        nc.tensor.matmul(
                            ps[:, 0, :],
                            wt[k][:, ct * 128 : (ct + 1) * 128],
                            xt[k][:, ns * SQB : (ns + 1) * SQB],
                            start=(k == 0),
                            stop=(k == NK - 1),
                        )
                    nc.vector.tensor_scalar_add(
                        qk_sb[ct][:, ns * SQB : (ns + 1) * SQB], ps[:, 0, :], bqk[ct]
                    )

            # v natural layout + ones column: v_sb[t] [128, HPC, 65];
            # [:, h, :64] = v for head h, [:, h, 64] = 1.0
            v_sb = []
            for t_i in range(NT):
                t = persist.tile(
                    [128, HPC, HD + 1], BF16, tag=f"v{t_i}", name=f"v{t_i}"
                )
                v_sb.append(t)
            for t_i in range(NT):
                psv = mm_pool.tile([128, 2, SQB], F32, tag="mm", name="ps_v")
                for k in range(NK):
                    nc.tensor.matmul(
                        psv[:, 0, 0:QC],
                        xt[k][:, t_i * 128 : (t_i + 1) * 128],
                        wt[k][:, 2 * QC : 3 * QC],
                        start=(k == 0),
                        stop=False,
                    )
                nc.tensor.matmul(psv[:, 0, 0:QC], ones_row, bv, start=False, stop=True)
                nc.vector.memset(v_sb[t_i][:, :, HD : HD + 1], 1.0)
                for h in range(HPC):
                    nc.vector.tensor_copy(
                        v_sb[t_i][:, h, 0:HD], psv[:, 0, h * HD : (h + 1) * HD]
                    )

            # ---- attention + c_proj + reduce-scatter, chunk by chunk ----
            for j in range(NJ):
                om = []  # merged per-pair c_proj lhsT tiles [128, SQB]
                for p in range(2):
                    om.append(
                        ot_pool.tile([128, SQB], BF16, tag="om", name=f"om{p}")
                    )
                for h in range(HPC):
                    qslice = qk_sb[h // 2][
                        (h % 2) * HD : (h % 2) * HD + HD, j * SQB : (j + 1) * SQB
                    ]
                    krow = qk_sb[2 + h // 2][(h % 2) * HD : (h % 2) * HD + HD, :]
                    expt = expt_pool.tile(
                        [128, NT, SQB], BF16, tag="expt", name="expt"
                    )
                    for t2 in range(NT // 2):
                        ps_s = mm_pool.tile([128, 2, SQB], F32, tag="mm", name="ps_s")
                        for u in range(2):
                            t_i = 2 * t2 + u
                            nc.tensor.matmul(
                                ps_s[:, u, :],
                                krow[:, t_i * 128 : (t_i + 1) * 128],
                                qslice,
                                start=True,
                                stop=True,
                            )
                        nc.scalar.activation(
                            expt[:, 2 * t2 : 2 * t2 + 2, :],
                            ps_s,
                            mybir.ActivationFunctionType.Exp,
                        )
                    pv = pv_pool.tile([HD + 1, SQB], F32, tag="pv", name="pv")
                    for t_i in range(NT):
                        nc.tensor.matmul(
                            pv,
                            v_sb[t_i][:, h, :],
                            expt[:, t_i, :],
                            start=(t_i == 0),
                            stop=(t_i == NT - 1),
                        )
                    rz = small_pool.tile([1, SQB], F32, tag="rz", name="rz")
                    nc.vector.reciprocal(rz, pv[HD : HD + 1, :])
                    bc = small_pool.tile([HD, SQB], F32, tag="bc", name="bc")
                    nc.gpsimd.partition_broadcast(bc, rz)
                    if h % 2 == 0:
                        nc.vector.tensor_mul(om[h // 2][0:HD, :], pv[0:HD, :], bc)
                    else:
                        o = ot_pool.tile([HD, SQB], BF16, tag="ot", name="ot")
                        nc.vector.tensor_mul(o, pv[0:HD, :], bc)
                        nc.sync.dma_start(out=om[h // 2][HD:128, :], in_=o)
                # c_proj on this chunk: rows j*SQB .. +SQB of partial (bf16)
                for m in range(SQB // 128):
                    for nch in range(2):
                        pc = mm_pool.tile([128, 2, SQB], F32, tag="mm", name="pc")
                        for p in range(2):
                            nc.tensor.matmul(
                                pc[:, 0, :],
                                om[p][:, m * 128 : (m + 1) * 128],
                                wp[p][:, nch * 512 : (nch + 1) * 512],
                                start=(p == 0),
                                stop=False,
                            )
                        nc.tensor.matmul(
                            pc[:, 0, :],
                            ones_row,
                            wp_bias[:, nch * 512 : (nch + 1) * 512],
                            start=False,
                            stop=True,
                        )
                        osb = osb_pool.tile([128, 512], BF16, tag="osb", name="osb")
                        nc.vector.tensor_copy(osb, pc[:, 0, :])
                        nc.sync.dma_start(
                            out=partial[
                                j * SQB + m * 128 : j * SQB + (m + 1) * 128,
                                nch * 512 : (nch + 1) * 512,
                            ],
                            in_=osb,
                        )
                # reduce-scatter this chunk (bf16, 1MB per core)
                nc.gpsimd.collective_compute(
                    "ReduceScatter",
                    mybir.AluOpType.add,
                    replica_groups=[[0, 1, 2, 3], [4, 5, 6, 7]],
                    ins=[partial[j * SQB : (j + 1) * SQB, :]],
                    outs=[rs_out[j].ap()],
                )
                # cast this chunk's slice to f32 and emit
                rcast = osb_pool.tile([128, D], BF16, tag="rcast", name="rcast")
                nc.sync.dma_start(out=rcast, in_=rs_out[j][:, :])
                rf32 = osb_pool.tile([128, D], F32, tag="rf32", name="rf32")
                nc.vector.tensor_copy(rf32, rcast)
                nc.sync.dma_start(
                    out=out_ext[j * 128 : (j + 1) * 128, :], in_=rf32
                )

    nc.compile()
    return nc


def make_in_maps(x, w_attn, b_attn, w_proj, b_proj):
    bf = ml_dtypes.bfloat16
    in_maps = []
    for c in range(8):
        b = c // TP
        g = c % TP
        cs = slice(g * QC, (g + 1) * QC)
        xT = np.ascontiguousarray(x[b].T).astype(bf)
        wqkv = np.concatenate(
            [w_attn[:, cs], w_attn[:, D:][:, cs], w_attn[:, 2 * D :][:, cs]], axis=1
        ).astype(bf)
        bqk = np.concatenate([b_attn[cs], b_attn[D:][cs]]).reshape(2 * QC, 1)
        bqk = np.ascontiguousarray(bqk, dtype=np.float32)
        bv = np.ascontiguousarray(b_attn[2 * D :][cs].reshape(1, QC).astype(bf))
        wpa = np.concatenate(
            [w_proj[cs, :], (b_proj / TP).reshape(1, D)], axis=0
        ).astype(bf)
        in_maps.append({"xT": xT, "wqkv": wqkv, "bqk": bqk, "bv": bv, "wpa": wpa})
    return in_maps


def assemble(results):
    # Chunk j's reduce-scatter gives core (group rank g) rows
    # j*SQB + g*128 .. +128; the kernel writes them to out rows j*128..,
    # so core c's "out" holds rows {j*SQB + g*128 + r} for j in 0..3.
    out = np.empty((B, S, D), np.float32)
    for c in range(8):
        b = c // TP
        g = c % TP
        o = results[c]["out"]
        for j in range(NJ):
            out[b, j * SQB + g * 128 : j * SQB + (g + 1) * 128, :] = o[
                j * 128 : (j + 1) * 128
            ]
    return out


def kernel(x, w_attn, b_attn, w_proj, b_proj):
    x = np.asarray(x, dtype=np.float32)
    w_attn = np.asarray(w_attn, dtype=np.float32)
    b_attn = np.asarray(b_attn, dtype=np.float32)
    w_proj = np.asarray(w_proj, dtype=np.float32)
    b_proj = np.asarray(b_proj, dtype=np.float32)
    if "nc" not in _CACHE:
        _CACHE["nc"] = build()
    nc = _CACHE["nc"]
    in_maps = make_in_maps(x, w_attn, b_attn, w_proj, b_proj)
    res = run_bass_kernel_spmd(nc, in_maps, core_ids=list(range(8)))
    return assemble(res.results)


# revision 22
# speedup vs baseline: 1.6268x; 1.1087x over previous
"""Distributed attention kernel for 8 TRN2 NeuronCores.

Problem: B=2, S=2048, D=1024, H=16 heads (hd=64), no causal mask, no
scaling.  out = softmax((x@Wq) (x@Wk)^T) (x@Wv) @ Wp + biases.

Sharding: DP=2 over batch x TP=4 over heads.  Core c handles batch c//4
and heads 4*(c%4) .. 4*(c%4)+3.  Each core computes its 4 heads'
attention plus the partial c_proj (rows of w_proj for its heads), then a
chunked bf16 ReduceScatter(add) over its 4-core group yields each core's
512-row slice of the final output.  The host reassembles [2,2048,1024].

All matmuls run in bf16 (inputs converted host-side); accumulation f32.
Softmax skips the max-subtraction (scores are O(+-20), exp is safe in
f32): probs = exp(s) / sum exp(s); the denominator comes free as the
65th row of the PV matmul via an appended ones-column on V.
"""

import sys

if "/opt/trn_rl_repo" not in sys.path:
    sys.path.insert(0, "/opt/trn_rl_repo")

import numpy as np
import ml_dtypes

import concourse.bass as bass
import concourse.mybir as mybir
from concourse import bacc
from concourse.tile import TileContext
from concourse.bass_utils import run_bass_kernel_spmd

BF16 = mybir.dt.bfloat16
F32 = mybir.dt.float32

B, S, D = 2, 2048, 1024
H = 16
HD = 64
TP = 4  # tensor-parallel group size (cores per batch)
HPC = H // TP  # heads per core = 4
QC = HPC * HD  # q (or k or v) columns per core = 256
SQB = 512  # sq chunk (free dim of scores/pv matmuls)
NJ = S // SQB  # 4 chunks
NT = S // 128  # 16 sk tiles
NK = D // 128  # 8 contraction tiles for the projections
SO = S // TP  # 512 output rows per core

_CACHE = {}


def build():
    nc = bacc.Bacc(num_devices=8)

    xT_ext = nc.declare_dram_parameter("xT", [D, S], BF16, isOutput=False)
    wqkv_ext = nc.declare_dram_parameter("wqkv", [D, 3 * QC], BF16, isOutput=False)
    bqk_ext = nc.declare_dram_parameter("bqk", [2 * QC, 1], F32, isOutput=False)
    bv_ext = nc.declare_dram_parameter("bv", [1, QC], BF16, isOutput=False)
    wpa_ext = nc.declare_dram_parameter("wpa", [QC + 1, D], BF16, isOutput=False)
    out_ext = nc.declare_dram_parameter("out", [SO, D], F32, isOutput=True)

    partial = nc.dram_tensor("partial", [S, D], BF16)
    rs_out = [nc.dram_tensor(f"rs_out{j}", [SQB // TP, D], BF16) for j in range(NJ)]

    with TileContext(nc) as tc:
        with (
            tc.tile_pool(name="persist", bufs=1) as persist,
            tc.tile_pool(name="expt_pool", bufs=3) as expt_pool,
            tc.tile_pool(name="mm", bufs=3, space="PSUM") as mm_pool,
            tc.tile_pool(name="pv", bufs=2, space="PSUM") as pv_pool,
            tc.tile_pool(name="small", bufs=4) as small_pool,
            tc.tile_pool(name="ot", bufs=4) as ot_pool,
            tc.tile_pool(name="osb", bufs=6) as osb_pool,
        ):
            # ---- load persistent tiles ----
            # Interleave w/x loads in k order so the first qkv matmul
            # (k=0) can start as soon as the first pair lands.
            xt = []
            wt = []
            for k in range(NK):
                tw = persist.tile([128, 3 * QC], BF16, tag=f"wt{k}", name=f"wt{k}")
                nc.sync.dma_start(out=tw, in_=wqkv_ext[k * 128 : (k + 1) * 128, :])
                wt.append(tw)
                tx = persist.tile([128, S], BF16, tag=f"xt{k}", name=f"xt{k}")
                nc.sync.dma_start(out=tx, in_=xT_ext[k * 128 : (k + 1) * 128, :])
                xt.append(tx)
            wp = []
            for p in range(2):
                t = persist.tile([128, D], BF16, tag=f"wp{p}", name=f"wp{p}")
                nc.sync.dma_start(out=t, in_=wpa_ext[p * 128 : (p + 1) * 128, :])
                wp.append(t)
            wp_bias = persist.tile([1, D], BF16, tag="wpb", name="wpb")
            nc.sync.dma_start(out=wp_bias, in_=wpa_ext[2 * 128 : 2 * 128 + 1, :])
            bqk = []
            for k in range(4):
                t = persist.tile([128, 1], F32, tag=f"bqk{k}", name=f"bqk{k}")
                nc.sync.dma_start(out=t, in_=bqk_ext[k * 128 : (k + 1) * 128, :])
                bqk.append(t)
            bv = persist.tile([1, QC], BF16, tag="bv", name="bv")
            nc.sync.dma_start(out=bv, in_=bv_ext[:, :])
            ones_row = persist.tile([1, 128], BF16, tag="ones", name="ones")
            nc.vector.memset(ones_row, 1.0)

            # ---- QKV projection ----
            # q/k transposed layout: qk_sb[ct] [128, S], ct 0-1 = q cols,
            # ct 2-3 = k cols; head h lives on partitions (h%2)*64 of
            # tile h//2 (+2 for k).  Emit k first so attention can start
            # before the q tiles for later chunks are done.
            qk_sb = [
                persist.tile([128, S], BF16, tag=f"qk{ct}", name=f"qk{ct}")
                for ct in range(4)
            ]
            def qkv_col_tile(ct, ns):
                ps = mm_pool.tile([128, 2, SQB], F32, tag="mm", name="ps_qkv")
                for k in range(NK):
                    nc.tensor.matmul(
                        ps[:, 0, :],
                        wt[k][:, ct * 128 : (ct + 1) * 128],
                        xt[k][:, ns * SQB : (ns + 1) * SQB],
                        start=(k == 0),
                        stop=(k == NK - 1),
                    )
                nc.vector.tensor_scalar_add(
                    qk_sb[ct][:, ns * SQB : (ns + 1) * SQB], ps[:, 0, :], bqk[ct]
                )

            # k tiles first (attention needs the full kT), v next, then q
            # chunk-major so chunk 0's attention can start early.
            for ct in (2, 3):
                for ns in range(NJ):
                    qkv_col_tile(ct, ns)

            # v natural layout + ones column: v_sb[t] [128, HPC, 65];
            # [:, h, :64] = v for head h, [:, h, 64] = 1.0
            v_sb = []
            for t_i in range(NT):
                t = persist.tile(
                    [128, HPC, HD + 1], BF16, tag=f"v{t_i}", name=f"v{t_i}"
                )
                v_sb.append(t)
            for t_i in range(NT):
                psv = mm_pool.tile([128, 2, SQB], F32, tag="mm", name="ps_v")
                for k in range(NK):
                    nc.tensor.matmul(
                        psv[:, 0, 0:QC],
                        xt[k][:, t_i * 128 : (t_i + 1) * 128],
                        wt[k][:, 2 * QC : 3 * QC],
                        start=(k == 0),
                        stop=False,
                    )
                nc.tensor.matmul(psv[:, 0, 0:QC], ones_row, bv, start=False, stop=True)
                nc.vector.memset(v_sb[t_i][:, :, HD : HD + 1], 1.0)
                for h in range(HPC):
                    nc.vector.tensor_copy(
                        v_sb[t_i][:, h, 0:HD], psv[:, 0, h * HD : (h + 1) * HD]
                    )
            # q tiles, chunk-major
            for ns in range(NJ):
                for ct in (0, 1):
                    qkv_col_tile(ct, ns)

            # ---- attention + c_proj + reduce-scatter, head-pipelined ----
            # Stage A(j,h): scores + exp.  Stage B(j,h): pv + normalize.
            # Emission order keeps the in-order PE queue dense:
            #   A(j,0) A(j,1) B(j,0) [cproj(j-1)] A(j,2) B(j,1) A(j,3)
            #   B(j,2) A(j+1,0) B(j,3) ...
            # cproj/RS for chunk j-1 land between chunk j's stages so the
            # PE never head-of-line blocks on the normalize chain.

            def stage_a(j, h):
                qslice = qk_sb[h // 2][
                    (h % 2) * HD : (h % 2) * HD + HD, j * SQB : (j + 1) * SQB
                ]
                krow = qk_sb[2 + h // 2][(h % 2) * HD : (h % 2) * HD + HD, :]
                expt = expt_pool.tile([128, NT, SQB], BF16, tag="expt", name="expt")
                for t2 in range(NT // 2):
                    ps_s = mm_pool.tile([128, 2, SQB], F32, tag="mm", name="ps_s")
                    for u in range(2):
                        t_i = 2 * t2 + u
                        nc.tensor.matmul(
                            ps_s[:, u, :],
                            krow[:, t_i * 128 : (t_i + 1) * 128],
                            qslice,
                            start=True,
                            stop=True,
                        )
                    nc.scalar.activation(
                        expt[:, 2 * t2 : 2 * t2 + 2, :],
                        ps_s,
                        mybir.ActivationFunctionType.Exp,
                    )
                return expt

            def stage_b(j, h, expt, om):
                pv = pv_pool.tile([HD + 1, SQB], F32, tag="pv", name=f"pv{h}")
                for t_i in range(NT):
                    nc.tensor.matmul(
                        pv,
                        v_sb[t_i][:, h, :],
                        expt[:, t_i, :],
                        start=(t_i == 0),
                        stop=(t_i == NT - 1),
                    )
                rz = small_pool.tile([1, SQB], F32, tag="rz", name="rz")
                nc.vector.reciprocal(rz, pv[HD : HD + 1, :])
                bc = small_pool.tile([HD, SQB], F32, tag="bc", name="bc")
                nc.gpsimd.partition_broadcast(bc, rz)
                if h % 2 == 0:
                    nc.vector.tensor_mul(om[h // 2][0:HD, :], pv[0:HD, :], bc)
                else:
                    o = ot_pool.tile([HD, SQB], BF16, tag="ot", name="ot")
                    nc.vector.tensor_mul(o, pv[0:HD, :], bc)
                    nc.sync.dma_start(out=om[h // 2][HD:128, :], in_=o)

            def cproj_rs(j, om):
                # c_proj chunk j -> partial rows, then chunked RS + emit
                for m in range(SQB // 128):
                    for nch in range(2):
                        pc = mm_pool.tile([128, 2, SQB], F32, tag="mm", name="pc")
                        for p in range(2):
                            nc.tensor.matmul(
                                pc[:, 0, :],
                                om[p][:, m * 128 : (m + 1) * 128],
                                wp[p][:, nch * 512 : (nch + 1) * 512],
                                start=(p == 0),
                                stop=False,
                            )
                        nc.tensor.matmul(
                            pc[:, 0, :],
                            ones_row,
                            wp_bias[:, nch * 512 : (nch + 1) * 512],
                            start=False,
                            stop=True,
                        )
                        osb = osb_pool.tile([128, 512], BF16, tag="osb", name="osb")
                        nc.vector.tensor_copy(osb, pc[:, 0, :])
                        nc.sync.dma_start(
                            out=partial[
                                j * SQB + m * 128 : j * SQB + (m + 1) * 128,
                                nch * 512 : (nch + 1) * 512,
                            ],
                            in_=osb,
                        )
                nc.gpsimd.collective_compute(
                    "ReduceScatter",
                    mybir.AluOpType.add,
                    replica_groups=[[0, 1, 2, 3], [4, 5, 6, 7]],
                    ins=[partial[j * SQB : (j + 1) * SQB, :]],
                    outs=[rs_out[j].ap()],
                )
                rcast = osb_pool.tile([128, D], BF16, tag="rcast", name="rcast")
                nc.sync.dma_start(out=rcast, in_=rs_out[j][:, :])
                rf32 = osb_pool.tile([128, D], F32, tag="rf32", name="rf32")
                nc.vector.tensor_copy(rf32, rcast)
                nc.sync.dma_start(out=out_ext[j * 128 : (j + 1) * 128, :], in_=rf32)

            om_of = {}
            prev_expt = None  # (h, expt) pending stage_b within current chunk
            for j in range(NJ):
                om_of[j] = [
                    ot_pool.tile([128, SQB], BF16, tag="om", name=f"om{p}")
                    for p in range(2)
                ]
                for h in range(HPC):
                    expt = stage_a(j, h)
                    if prev_expt is not None:
                        pj, ph, pe = prev_expt
                        stage_b(pj, ph, pe, om_of[pj])
                    prev_expt = (j, h, expt)
                    if h == 1 and j >= 1:
                        cproj_rs(j - 1, om_of[j - 1])
            # drain: B for the last head, then final cproj/RS
            pj, ph, pe = prev_expt
            stage_b(pj, ph, pe, om_of[pj])
            cproj_rs(NJ - 1, om_of[NJ - 1])

    nc.compile()
    return nc


def make_in_maps(x, w_attn, b_attn, w_proj, b_proj):
    bf = ml_dtypes.bfloat16
    in_maps = []
    for c in range(8):
        b = c // TP
        g = c % TP
        cs = slice(g * QC, (g + 1) * QC)
        xT = np.ascontiguousarray(x[b].T).astype(bf)
        wqkv = np.concatenate(
            [w_attn[:, cs], w_attn[:, D:][:, cs], w_attn[:, 2 * D :][:, cs]], axis=1
        ).astype(bf)
        bqk = np.concatenate([b_attn[cs], b_attn[D:][cs]]).reshape(2 * QC, 1)
        bqk = np.ascontiguousarray(bqk, dtype=np.float32)
        bv = np.ascontiguousarray(b_attn[2 * D :][cs].reshape(1, QC).astype(bf))
        wpa = np.concatenate(
            [w_proj[cs, :], (b_proj / TP).reshape(1, D)], axis=0
        ).astype(bf)
        in_maps.append({"xT": xT, "wqkv": wqkv, "bqk": bqk, "bv": bv, "wpa": wpa})
    return in_maps


def assemble(results):
    # Chunk j's reduce-scatter gives core (group rank g) rows
    # j*SQB + g*128 .. +128; the kernel writes them to out rows j*128..,
    # so core c's "out" holds rows {j*SQB + g*128 + r} for j in 0..3.
    out = np.empty((B, S, D), np.float32)
    for c in range(8):
        b = c // TP
        g = c % TP
        o = results[c]["out"]
        for j in range(NJ):
            out[b, j * SQB + g * 128 : j * SQB + (g + 1) * 128, :] = o[
                j * 128 : (j + 1) * 128
            ]
    return out


def kernel(x, w_attn, b_attn, w_proj, b_proj):
    x = np.asarray(x, dtype=np.float32)
    w_attn = np.asarray(w_attn, dtype=np.float32)
    b_attn = np.asarray(b_attn, dtype=np.float32)
    w_proj = np.asarray(w_proj, dtype=np.float32)
    b_proj = np.asarray(b_proj, dtype=np.float32)
    if "nc" not in _CACHE:
        _CACHE["nc"] = build()
    nc = _CACHE["nc"]
    in_maps = make_in_maps(x, w_attn, b_attn, w_proj, b_proj)
    res = run_bass_kernel_spmd(nc, in_maps, core_ids=list(range(8)))
    return assemble(res.results)
